# revision 2
# baseline (speedup 1.0000x reference)
"""Trainium2 Bass kernel for nn_CrossAttention (gnn_message_passing).

Math (per batch b):
  q = (q_w/sqrt(D)) @ pcd            (C, N)
  k = k_w @ neighbors                (C, N, K)
  v = v_w @ neighbors                (C, N, K)
  energy[h,n,k] = sum_d q[h*8+d,n] * k[h*8+d,n,k]
  attn = softmax_k(energy)           (exp without max-subtraction; |energy| is O(5))
  x[h*8+d, n] = sum_k attn[h,n,k] * v[h*8+d,n,k]

Mapping (per core, N sharded 8 ways -> NC=1024 points):
  - batches stacked in pairs on the 128 partitions: partition = (bb, c), bb in {0,1}
  - PE: block-diag weight stationaries for q/k/v projections (bf16 in, fp32 accum)
  - PE: block-ones stationary reduces over d AND replicates the result across d
    in one matmul -> energies land replicated, so exp's output is directly
    aligned with v for the attn*v multiply.
  - ACT: exp (PSUM -> SBUF bf16 eviction for free)
  - DVE: q*k multiply, attn*v multiply, pairwise-halving sum trees over K,
    reciprocal, final normalization.
"""

import numpy as np
import ml_dtypes

import concourse.bass as bass
import concourse.tile as tile
from concourse import bacc, mybir
from concourse.bass_utils import run_bass_kernel_spmd

BF16 = mybir.dt.bfloat16
F32 = mybir.dt.float32

B, CIN, N, K = 4, 64, 8192, 32
COUT, H = 64, 8
D = COUT // H
NCORES = 8
NC = N // NCORES  # points per core

_nc_cache = {}


def build_nc(NC=NC, S=64, R=1, attn_f32=False):
    """Build the per-core Bass program.

    NC: points per core, S: strip size (points per DVE/ACT macro-op),
    R: on-device repeat count (for timing), attn_f32: keep attn in fp32.
    """
    key = (NC, S, R, attn_f32)
    if key in _nc_cache:
        return _nc_cache[key]

    PTS = 16           # points per matmul chunk (16*32 = 512 cols = 1 PSUM bank)
    NCHUNK = S // PTS  # matmul chunks per strip
    assert S % PTS == 0 and NC % S == 0
    ADT = F32 if attn_f32 else BF16

    nc = bacc.Bacc("TRN2", target_bir_lowering=False, debug=False,
                   num_devices=NCORES)
    nbp = nc.dram_tensor("nbp", [2, 128, NC, K], BF16, kind="ExternalInput").ap()
    pcdp = nc.dram_tensor("pcdp", [2, 128, NC], BF16, kind="ExternalInput").ap()
    wq_d = nc.dram_tensor("wq", [128, 128], BF16, kind="ExternalInput").ap()
    wk_d = nc.dram_tensor("wk", [128, 128], BF16, kind="ExternalInput").ap()
    wv_d = nc.dram_tensor("wv", [128, 128], BF16, kind="ExternalInput").ap()
    wo_d = nc.dram_tensor("wones", [128, 128], BF16, kind="ExternalInput").ap()
    xout = nc.dram_tensor("xout", [2, 128, NC], F32, kind="ExternalOutput").ap()

    with tile.TileContext(nc) as tc:
        with (
            tc.tile_pool(name="const", bufs=1) as cpool,
            tc.tile_pool(name="io", bufs=3) as iopool,
            tc.tile_pool(name="work", bufs=2) as wpool,
            tc.tile_pool(name="acc", bufs=2) as apool,
            tc.tile_pool(name="ps", bufs=2, space="PSUM") as pspool,
        ):
            wq_t = cpool.tile([128, 128], BF16, tag="wq")
            nc.sync.dma_start(wq_t[:], wq_d[:])
            wk_t = cpool.tile([128, 128], BF16, tag="wk")
            nc.sync.dma_start(wk_t[:], wk_d[:])
            wv_t = cpool.tile([128, 128], BF16, tag="wv")
            nc.sync.dma_start(wv_t[:], wv_d[:])
            wo_t = cpool.tile([128, 128], BF16, tag="wo")
            nc.sync.dma_start(wo_t[:], wo_d[:])

            def tree_sum(src, tag):
                """src (128, S, 32) -> (128, S) fp32 sum over last axis."""
                cur = src
                w = 16
                while w >= 2:
                    nxt = wpool.tile([128, S, w], ADT, tag=f"{tag}{w}")
                    nc.vector.tensor_add(nxt[:], cur[:, :, 0:w], cur[:, :, w:2 * w])
                    cur = nxt
                    w //= 2
                outf = wpool.tile([128, S], F32, tag=f"{tag}1")
                nc.vector.tensor_add(outf[:], cur[:, :, 0], cur[:, :, 1])
                return outf

            def body():
                for pp in range(2):
                    pcd_t = iopool.tile([128, NC], BF16, tag="pcd")
                    nc.sync.dma_start(pcd_t[:], pcdp[pp])
                    q_sb = apool.tile([128, NC], F32, tag="q")
                    QCH = min(512, NC)
                    for h in range(NC // QCH):
                        q_ps = pspool.tile([128, S, K], F32, tag="ps")
                        qp = q_ps[:, 0:QCH // K, :]  # (128, QCH//K, 32) = QCH cols
                        nc.tensor.matmul(qp, wq_t[:],
                                         pcd_t[:, h * QCH:(h + 1) * QCH],
                                         start=True, stop=True)
                        nc.vector.tensor_copy(q_sb[:, h * QCH:(h + 1) * QCH], qp)
                    x_strip = apool.tile([128, NC], F32, tag="xs")
                    for s in range(NC // S):
                        n0 = s * S
                        nb_t = iopool.tile([128, S, K], BF16, tag="nb")
                        nc.sync.dma_start(nb_t[:], nbp[pp, :, n0:n0 + S, :])
                        # phase A: k projection + q*k
                        k_ps = pspool.tile([128, S, K], F32, tag="ps")
                        for j in range(NCHUNK):
                            nc.tensor.matmul(
                                k_ps[:, j * PTS:(j + 1) * PTS, :], wk_t[:],
                                nb_t[:, j * PTS:(j + 1) * PTS, :],
                                start=True, stop=True)
                        prod = wpool.tile([128, S, K], BF16, tag="prod")
                        qb = q_sb[:, n0:n0 + S].unsqueeze(2).broadcast_to([128, S, K])
                        nc.vector.tensor_mul(prod[:], k_ps[:], qb)
                        # phase B: reduce over d + replicate -> exp
                        e_ps = pspool.tile([128, S, K], F32, tag="ps")
                        for j in range(NCHUNK):
                            nc.tensor.matmul(
                                e_ps[:, j * PTS:(j + 1) * PTS, :], wo_t[:],
                                prod[:, j * PTS:(j + 1) * PTS, :],
                                start=True, stop=True)
                        attn = wpool.tile([128, S, K], ADT, tag="attn")
                        nc.scalar.activation(attn[:], e_ps[:],
                                             mybir.ActivationFunctionType.Exp)
                        # phase C: v projection + attn*v
                        v_ps = pspool.tile([128, S, K], F32, tag="ps")
                        for j in range(NCHUNK):
                            nc.tensor.matmul(
                                v_ps[:, j * PTS:(j + 1) * PTS, :], wv_t[:],
                                nb_t[:, j * PTS:(j + 1) * PTS, :],
                                start=True, stop=True)
                        prod2 = wpool.tile([128, S, K], ADT, tag="prod2")
                        nc.vector.tensor_mul(prod2[:], v_ps[:], attn[:])
                        x_un = tree_sum(prod2, "tx")
                        den = tree_sum(attn, "td")
                        rden = wpool.tile([128, S], F32, tag="rden")
                        nc.vector.reciprocal(rden[:], den[:])
                        nc.vector.tensor_mul(x_strip[:, n0:n0 + S], x_un[:], rden[:])
                    nc.sync.dma_start(xout[pp], x_strip[:])

            if R == 1:
                body()
            else:
                with tc.For_i(0, R, 1):
                    body()

    nc.compile()
    _nc_cache[key] = nc
    return nc


def prep_inputs(pcd, neighbors, q_w, k_w, v_w, NC=NC):
    """Host-side prep: cast to bf16, pair-stack batches, build stationaries."""
    bf = ml_dtypes.bfloat16
    s = 1.0 / np.sqrt(np.float32(D))
    qwT = (q_w.astype(np.float32) * s).T.astype(bf)  # (c, hd)
    kwT = k_w.T.astype(bf)
    vwT = v_w.T.astype(bf)

    def blockdiag(m):
        z = np.zeros((128, 128), dtype=bf)
        z[:64, :64] = m
        z[64:, 64:] = m
        return z

    wq = blockdiag(qwT)
    wk = blockdiag(kwT)
    wv = blockdiag(vwT)
    blk = np.kron(np.eye(H, dtype=np.float32), np.ones((D, D), np.float32))
    wones = blockdiag(blk.astype(bf))

    nbs = neighbors.reshape(2, 2 * CIN, N, K)    # (pair, bb*64+c, n, k)
    pcds = pcd.reshape(2, 2 * CIN, N)
    ncores = N // NC
    in_maps = []
    for i in range(ncores):
        sl = slice(i * NC, (i + 1) * NC)
        in_maps.append({
            "nbp": np.ascontiguousarray(nbs[:, :, sl, :]).astype(bf),
            "pcdp": np.ascontiguousarray(pcds[:, :, sl]).astype(bf),
            "wq": wq, "wk": wk, "wv": wv, "wones": wones,
        })
    return in_maps


def assemble_output(results, NC=NC):
    ncores = len(results)
    out = np.empty((B, COUT, N), dtype=np.float32)
    for i, r in enumerate(results):
        x = r["xout"].reshape(B, COUT, NC)  # (2,128,NC) -> (4,64,NC)
        out[:, :, i * NC:(i + 1) * NC] = x
    return out


def kernel(pcd, neighbors, q_w, k_w, v_w):
    pcd = np.asarray(pcd, dtype=np.float32)
    neighbors = np.asarray(neighbors, dtype=np.float32)
    nc = build_nc()
    in_maps = prep_inputs(pcd, neighbors, q_w, k_w, v_w)
    res = run_bass_kernel_spmd(nc, in_maps, core_ids=list(range(NCORES)))
    return assemble_output(res.results)


if __name__ == "__main__":
    rng = np.random.default_rng(0)
    ins = {
        "pcd": rng.standard_normal((B, CIN, N), dtype=np.float32),
        "neighbors": rng.standard_normal((B, CIN, N, K), dtype=np.float32),
        "q_w": (rng.standard_normal((COUT, CIN), dtype=np.float32) / 8.0),
        "k_w": (rng.standard_normal((COUT, CIN), dtype=np.float32) / 8.0),
        "v_w": (rng.standard_normal((COUT, CIN), dtype=np.float32) / 8.0),
    }
    out = kernel(**ins)
    print("kernel output", out.shape, out.dtype)


# revision 11
# speedup vs baseline: 8.7155x; 8.7155x over previous
"""Trainium2 Bass kernel for nn_CrossAttention (gnn_message_passing).

Math (per batch b):
  q = (q_w/sqrt(D)) @ pcd            (C, N)
  k = k_w @ neighbors                (C, N, K)
  v = v_w @ neighbors                (C, N, K)
  energy[h,n,k] = sum_d q[h*8+d,n] * k[h*8+d,n,k]
  attn = softmax_k(energy)           (exp without max-subtraction; |energy| is O(5))
  x[h*8+d, n] = sum_k attn[h,n,k] * v[h*8+d,n,k]

Mapping (per core, N sharded 8 ways -> NC=1024 points):
  - batches stacked in pairs on the 128 partitions: partition = (bb, c), bb in {0,1}
  - PE: block-diag weight stationaries for q/k/v projections (bf16 in, fp32 accum)
  - PE: block-ones stationary reduces over d AND replicates the result across d
    in one matmul -> energies land replicated, so exp's output is directly
    aligned with v for the attn*v multiply.
  - ACT: exp (PSUM -> SBUF bf16 eviction for free)
  - DVE: q*k multiply, attn*v multiply, pairwise-halving sum trees over K,
    reciprocal, final normalization.
"""

import numpy as np
import ml_dtypes

import concourse.bass as bass
import concourse.tile as tile
from concourse import bacc, mybir
from concourse.bass_utils import run_bass_kernel_spmd

BF16 = mybir.dt.bfloat16
F32 = mybir.dt.float32

B, CIN, N, K = 4, 64, 8192, 32
COUT, H = 64, 8
D = COUT // H
NCORES = 8
NC = N // NCORES  # points per core

_nc_cache = {}


def build_nc(NC=NC, S=64, R=1, attn_f32=False, psum_bufs=2):
    """Build the per-core Bass program.

    NC: points per core, S: strip size (points per DVE/ACT macro-op),
    R: on-device repeat count (for timing), attn_f32: keep attn in fp32.
    """
    key = (NC, S, R, attn_f32, psum_bufs)
    if key in _nc_cache:
        return _nc_cache[key]

    PTS = min(16, S)   # points per matmul chunk (16*32 = 512 cols = 1 PSUM bank)
    NCHUNK = S // PTS  # matmul chunks per strip
    assert S % PTS == 0 and NC % S == 0
    ADT = F32 if attn_f32 else BF16

    nc = bacc.Bacc("TRN2", target_bir_lowering=False, debug=False,
                   num_devices=NCORES)
    nbp = nc.dram_tensor("nbp", [2, 128, NC, K], BF16, kind="ExternalInput").ap()
    pcdp = nc.dram_tensor("pcdp", [2, 128, NC], BF16, kind="ExternalInput").ap()
    wq_d = nc.dram_tensor("wq", [128, 128], BF16, kind="ExternalInput").ap()
    wk_d = nc.dram_tensor("wk", [128, 128], BF16, kind="ExternalInput").ap()
    wv_d = nc.dram_tensor("wv", [128, 128], BF16, kind="ExternalInput").ap()
    wo_d = nc.dram_tensor("wones", [128, 128], BF16, kind="ExternalInput").ap()
    xout = nc.dram_tensor("xout", [2, 128, NC], F32, kind="ExternalOutput").ap()

    with tile.TileContext(nc) as tc:
        with (
            tc.tile_pool(name="const", bufs=1) as cpool,
            tc.tile_pool(name="io", bufs=3) as iopool,
            tc.tile_pool(name="work", bufs=2) as wpool,
            tc.tile_pool(name="acc", bufs=2) as apool,
            tc.tile_pool(name="ps", bufs=psum_bufs, space="PSUM") as pspool,
        ):
            wq_t = cpool.tile([128, 128], BF16, tag="wq")
            nc.sync.dma_start(wq_t[:], wq_d[:])
            wk_t = cpool.tile([128, 128], BF16, tag="wk")
            nc.sync.dma_start(wk_t[:], wk_d[:])
            wv_t = cpool.tile([128, 128], BF16, tag="wv")
            nc.sync.dma_start(wv_t[:], wv_d[:])
            wo_t = cpool.tile([128, 128], BF16, tag="wo")
            nc.sync.dma_start(wo_t[:], wo_d[:])

            def tree_sum(src, SB, tag):
                """src (128, SB, 32) -> (128, SB) fp32 sum over last axis."""
                cur = src
                w = 16
                while w >= 2:
                    nxt = wpool.tile([128, SB, w], ADT, tag=f"{tag}{w}")
                    nc.vector.tensor_add(nxt[:], cur[:, :, 0:w], cur[:, :, w:2 * w])
                    cur = nxt
                    w //= 2
                outf = wpool.tile([128, SB], F32, tag=f"{tag}1")
                nc.vector.tensor_add(outf[:], cur[:, :, 0], cur[:, :, 1])
                return outf

            def body():
                for pp in range(2):
                    pcd_t = iopool.tile([128, NC], BF16, tag="pcd")
                    nc.sync.dma_start(pcd_t[:], pcdp[pp])
                    q_sb = apool.tile([128, NC], F32, tag="q")
                    QCH = min(512, NC)
                    for h in range(NC // QCH):
                        q_ps = pspool.tile([128, S, K], F32, tag="ps")
                        qp = q_ps[:, 0:QCH // K, :]  # (128, QCH//K, 32) = QCH cols
                        nc.tensor.matmul(qp, wq_t[:],
                                         pcd_t[:, h * QCH:(h + 1) * QCH],
                                         start=True, stop=True)
                        nc.vector.tensor_copy(q_sb[:, h * QCH:(h + 1) * QCH], qp)
                    x_strip = apool.tile([128, NC], F32, tag="xs")
                    HNC = NC // 2 if NC >= 128 else NC  # points per nb load
                    SB = min(128, NC)  # tree/normalization batch (points)
                    assert SB % S == 0
                    # software-pipelined: strip s front half (projections,
                    # q*k, energies, exp) is emitted one step ahead of strip
                    # s's back half (attn*v, trees) to keep DVE's in-order
                    # queue from head-of-line blocking on exp.
                    pend = None

                    def back_half(st):
                        # per-strip: attn*v, then tree levels 16->4 for both
                        # sums into the SB-wide level-4 buffers; per-SB: the
                        # remaining levels + reciprocal + normalize. Keeps the
                        # DVE work smooth instead of a burst per SB block.
                        v_sb, attn_full, n0, tx4, td4 = st
                        o = n0 % SB
                        attn = attn_full[:, o:o + S, :]
                        prod2 = wpool.tile([128, S, K], ADT, tag="prod2")
                        nc.vector.tensor_mul(prod2[:], v_sb[:], attn)
                        for src_t, t4 in ((prod2, tx4), (attn, td4)):
                            cur = src_t
                            w = 16
                            while w >= 8:
                                nxt = wpool.tile([128, S, w], ADT, tag=f"s{t4.name[:3]}{w}")
                                nc.vector.tensor_add(nxt[:], cur[:, :, 0:w],
                                                     cur[:, :, w:2 * w])
                                cur = nxt
                                w //= 2
                            nc.vector.tensor_add(t4[:, o:o + S, :],
                                                 cur[:, :, 0:4], cur[:, :, 4:8])
                        if o + S == SB:
                            nb0 = n0 + S - SB
                            outs = []
                            for t4 in (tx4, td4):
                                t2 = wpool.tile([128, SB, 2], ADT, tag=f"f2{t4.name[:3]}")
                                nc.vector.tensor_add(t2[:], t4[:, :, 0:2], t4[:, :, 2:4])
                                t1 = wpool.tile([128, SB], F32, tag=f"f1{t4.name[:3]}")
                                nc.vector.tensor_add(t1[:], t2[:, :, 0], t2[:, :, 1])
                                outs.append(t1)
                            x_un, den = outs
                            rden = wpool.tile([128, SB], F32, tag="rden")
                            nc.vector.reciprocal(rden[:], den[:])
                            nc.vector.tensor_mul(x_strip[:, nb0:nb0 + SB],
                                                 x_un[:], rden[:])

                    for s in range(NC // S):
                        n0 = s * S
                        if n0 % HNC == 0:
                            nbh = iopool.tile([128, HNC, K], BF16, tag="nbh")
                            nc.sync.dma_start(nbh[:], nbp[pp, :, n0:n0 + HNC, :])
                        nb_t = nbh[:, n0 % HNC:n0 % HNC + S, :]
                        # v projection, ScalarE evicts to SBUF bf16
                        v_ps = pspool.tile([128, S, K], F32, tag="ps")
                        for j in range(NCHUNK):
                            nc.tensor.matmul(
                                v_ps[:, j * PTS:(j + 1) * PTS, :], wv_t[:],
                                nb_t[:, j * PTS:(j + 1) * PTS, :],
                                start=True, stop=True)
                        v_sb = wpool.tile([128, S, K], ADT, tag="vsb")
                        nc.scalar.copy(v_sb[:], v_ps[:])
                        # k projection + q*k
                        k_ps = pspool.tile([128, S, K], F32, tag="ps")
                        for j in range(NCHUNK):
                            nc.tensor.matmul(
                                k_ps[:, j * PTS:(j + 1) * PTS, :], wk_t[:],
                                nb_t[:, j * PTS:(j + 1) * PTS, :],
                                start=True, stop=True)
                        prod = wpool.tile([128, S, K], BF16, tag="prod")
                        qb = q_sb[:, n0:n0 + S].unsqueeze(2).broadcast_to([128, S, K])
                        nc.vector.tensor_mul(prod[:], k_ps[:], qb)
                        # reduce over d + replicate -> exp
                        e_ps = pspool.tile([128, S, K], F32, tag="ps")
                        for j in range(NCHUNK):
                            nc.tensor.matmul(
                                e_ps[:, j * PTS:(j + 1) * PTS, :], wo_t[:],
                                prod[:, j * PTS:(j + 1) * PTS, :],
                                start=True, stop=True)
                        if n0 % SB == 0:
                            attn_b = wpool.tile([128, SB, K], ADT, tag="attn")
                            tx4 = wpool.tile([128, SB, 4], ADT, tag="tx4", name="tx4")
                            td4 = wpool.tile([128, SB, 4], ADT, tag="td4", name="td4")
                        o = n0 % SB
                        nc.scalar.activation(attn_b[:, o:o + S, :], e_ps[:],
                                             mybir.ActivationFunctionType.Exp)
                        if pend is not None:
                            back_half(pend)
                        pend = (v_sb, attn_b, n0, tx4, td4)
                    back_half(pend)
                    nc.sync.dma_start(xout[pp], x_strip[:])

            if R == 1:
                body()
            else:
                with tc.For_i(0, R, 1):
                    body()

    nc.compile()
    _nc_cache[key] = nc
    return nc


def prep_inputs(pcd, neighbors, q_w, k_w, v_w, NC=NC):
    """Host-side prep: cast to bf16, pair-stack batches, build stationaries."""
    bf = ml_dtypes.bfloat16
    s = 1.0 / np.sqrt(np.float32(D))
    qwT = (q_w.astype(np.float32) * s).T.astype(bf)  # (c, hd)
    kwT = k_w.T.astype(bf)
    vwT = v_w.T.astype(bf)

    def blockdiag(m):
        z = np.zeros((128, 128), dtype=bf)
        z[:64, :64] = m
        z[64:, 64:] = m
        return z

    wq = blockdiag(qwT)
    wk = blockdiag(kwT)
    wv = blockdiag(vwT)
    blk = np.kron(np.eye(H, dtype=np.float32), np.ones((D, D), np.float32))
    wones = blockdiag(blk.astype(bf))

    nbs = neighbors.reshape(2, 2 * CIN, N, K)    # (pair, bb*64+c, n, k)
    pcds = pcd.reshape(2, 2 * CIN, N)
    ncores = N // NC
    in_maps = []
    for i in range(ncores):
        sl = slice(i * NC, (i + 1) * NC)
        in_maps.append({
            "nbp": np.ascontiguousarray(nbs[:, :, sl, :]).astype(bf),
            "pcdp": np.ascontiguousarray(pcds[:, :, sl]).astype(bf),
            "wq": wq, "wk": wk, "wv": wv, "wones": wones,
        })
    return in_maps


def assemble_output(results, NC=NC):
    ncores = len(results)
    out = np.empty((B, COUT, N), dtype=np.float32)
    for i, r in enumerate(results):
        x = r["xout"].reshape(B, COUT, NC)  # (2,128,NC) -> (4,64,NC)
        out[:, :, i * NC:(i + 1) * NC] = x
    return out


def kernel(pcd, neighbors, q_w, k_w, v_w):
    pcd = np.asarray(pcd, dtype=np.float32)
    neighbors = np.asarray(neighbors, dtype=np.float32)
    nc = build_nc()
    in_maps = prep_inputs(pcd, neighbors, q_w, k_w, v_w)
    res = run_bass_kernel_spmd(nc, in_maps, core_ids=list(range(NCORES)))
    return assemble_output(res.results)


if __name__ == "__main__":
    rng = np.random.default_rng(0)
    ins = {
        "pcd": rng.standard_normal((B, CIN, N), dtype=np.float32),
        "neighbors": rng.standard_normal((B, CIN, N, K), dtype=np.float32),
        "q_w": (rng.standard_normal((COUT, CIN), dtype=np.float32) / 8.0),
        "k_w": (rng.standard_normal((COUT, CIN), dtype=np.float32) / 8.0),
        "v_w": (rng.standard_normal((COUT, CIN), dtype=np.float32) / 8.0),
    }
    out = kernel(**ins)
    print("kernel output", out.shape, out.dtype)


# revision 12
# speedup vs baseline: 8.7651x; 1.0057x over previous
"""Trainium2 Bass kernel for nn_CrossAttention (gnn_message_passing).

Math (per batch b):
  q = (q_w/sqrt(D)) @ pcd            (C, N)
  k = k_w @ neighbors                (C, N, K)
  v = v_w @ neighbors                (C, N, K)
  energy[h,n,k] = sum_d q[h*8+d,n] * k[h*8+d,n,k]
  attn = softmax_k(energy)           (exp without max-subtraction; |energy| is O(5))
  x[h*8+d, n] = sum_k attn[h,n,k] * v[h*8+d,n,k]

Mapping (per core, N sharded 8 ways -> NC=1024 points):
  - batches stacked in pairs on the 128 partitions: partition = (bb, c), bb in {0,1}
  - PE: block-diag weight stationaries for q/k/v projections (bf16 in, fp32 accum)
  - PE: block-ones stationary reduces over d AND replicates the result across d
    in one matmul -> energies land replicated, so exp's output is directly
    aligned with v for the attn*v multiply.
  - ACT: exp (PSUM -> SBUF bf16 eviction for free)
  - DVE: q*k multiply, attn*v multiply, pairwise-halving sum trees over K,
    reciprocal, final normalization.
"""

import numpy as np
import ml_dtypes

import concourse.bass as bass
import concourse.tile as tile
from concourse import bacc, mybir
from concourse.bass_utils import run_bass_kernel_spmd

BF16 = mybir.dt.bfloat16
F32 = mybir.dt.float32

B, CIN, N, K = 4, 64, 8192, 32
COUT, H = 64, 8
D = COUT // H
NCORES = 8
NC = N // NCORES  # points per core

_nc_cache = {}


def build_nc(NC=NC, S=64, R=1, attn_f32=False, psum_bufs=2):
    """Build the per-core Bass program.

    NC: points per core, S: strip size (points per DVE/ACT macro-op),
    R: on-device repeat count (for timing), attn_f32: keep attn in fp32.
    """
    key = (NC, S, R, attn_f32, psum_bufs)
    if key in _nc_cache:
        return _nc_cache[key]

    PTS = min(16, S)   # points per matmul chunk (16*32 = 512 cols = 1 PSUM bank)
    NCHUNK = S // PTS  # matmul chunks per strip
    assert S % PTS == 0 and NC % S == 0
    ADT = F32 if attn_f32 else BF16

    nc = bacc.Bacc("TRN2", target_bir_lowering=False, debug=False,
                   num_devices=NCORES)
    nbp = nc.dram_tensor("nbp", [2, 128, NC, K], BF16, kind="ExternalInput").ap()
    pcdp = nc.dram_tensor("pcdp", [2, 128, NC], BF16, kind="ExternalInput").ap()
    wq_d = nc.dram_tensor("wq", [128, 128], BF16, kind="ExternalInput").ap()
    wk_d = nc.dram_tensor("wk", [128, 128], BF16, kind="ExternalInput").ap()
    wv_d = nc.dram_tensor("wv", [128, 128], BF16, kind="ExternalInput").ap()
    wo_d = nc.dram_tensor("wones", [128, 128], BF16, kind="ExternalInput").ap()
    xout = nc.dram_tensor("xout", [2, 128, NC], F32, kind="ExternalOutput").ap()

    with tile.TileContext(nc) as tc:
        with (
            tc.tile_pool(name="const", bufs=1) as cpool,
            tc.tile_pool(name="io", bufs=3) as iopool,
            tc.tile_pool(name="work", bufs=2) as wpool,
            tc.tile_pool(name="acc", bufs=2) as apool,
            tc.tile_pool(name="ps", bufs=psum_bufs, space="PSUM") as pspool,
        ):
            wq_t = cpool.tile([128, 128], BF16, tag="wq")
            nc.sync.dma_start(wq_t[:], wq_d[:])
            wk_t = cpool.tile([128, 128], BF16, tag="wk")
            nc.sync.dma_start(wk_t[:], wk_d[:])
            wv_t = cpool.tile([128, 128], BF16, tag="wv")
            nc.sync.dma_start(wv_t[:], wv_d[:])
            wo_t = cpool.tile([128, 128], BF16, tag="wo")
            nc.sync.dma_start(wo_t[:], wo_d[:])

            def tree_sum(src, SB, tag):
                """src (128, SB, 32) -> (128, SB) fp32 sum over last axis."""
                cur = src
                w = 16
                while w >= 2:
                    nxt = wpool.tile([128, SB, w], ADT, tag=f"{tag}{w}")
                    nc.vector.tensor_add(nxt[:], cur[:, :, 0:w], cur[:, :, w:2 * w])
                    cur = nxt
                    w //= 2
                outf = wpool.tile([128, SB], F32, tag=f"{tag}1")
                nc.vector.tensor_add(outf[:], cur[:, :, 0], cur[:, :, 1])
                return outf

            def body():
                for pp in range(2):
                    pcd_t = iopool.tile([128, NC], BF16, tag="pcd")
                    nc.sync.dma_start(pcd_t[:], pcdp[pp])
                    q_sb = apool.tile([128, NC], F32, tag="q")
                    QCH = min(512, NC)
                    for h in range(NC // QCH):
                        q_ps = pspool.tile([128, S, K], F32, tag="ps")
                        qp = q_ps[:, 0:QCH // K, :]  # (128, QCH//K, 32) = QCH cols
                        nc.tensor.matmul(qp, wq_t[:],
                                         pcd_t[:, h * QCH:(h + 1) * QCH],
                                         start=True, stop=True)
                        nc.vector.tensor_copy(q_sb[:, h * QCH:(h + 1) * QCH], qp)
                    x_strip = apool.tile([128, NC], F32, tag="xs")
                    HNC = NC // 2 if NC >= 128 else NC  # points per nb load
                    SB = min(128, NC)  # tree/normalization batch (points)
                    assert SB % S == 0
                    # software-pipelined: strip s front half (projections,
                    # q*k, energies, exp) is emitted one step ahead of strip
                    # s's back half (attn*v, trees) to keep DVE's in-order
                    # queue from head-of-line blocking on exp.
                    pend = None

                    def back_half(st):
                        # per-strip: attn*v, then tree levels 16->4 for both
                        # sums into the SB-wide level-4 buffers; per-SB: the
                        # remaining levels + reciprocal + normalize. Keeps the
                        # DVE work smooth instead of a burst per SB block.
                        v_sb, attn_full, n0, tx4, td4 = st
                        o = n0 % SB
                        attn = attn_full[:, o:o + S, :]
                        prod2 = wpool.tile([128, S, K], ADT, tag="prod2")
                        nc.vector.tensor_mul(prod2[:], v_sb[:], attn)
                        for src_t, t4 in ((prod2, tx4), (attn, td4)):
                            cur = src_t
                            w = 16
                            while w >= 8:
                                nxt = wpool.tile([128, S, w], ADT, tag=f"s{t4.name[:3]}{w}")
                                nc.vector.tensor_add(nxt[:], cur[:, :, 0:w],
                                                     cur[:, :, w:2 * w])
                                cur = nxt
                                w //= 2
                            nc.vector.tensor_add(t4[:, o:o + S, :],
                                                 cur[:, :, 0:4], cur[:, :, 4:8])
                        if o + S == SB:
                            nb0 = n0 + S - SB
                            outs = []
                            for t4 in (tx4, td4):
                                t2 = wpool.tile([128, SB, 2], ADT, tag=f"f2{t4.name[:3]}")
                                nc.vector.tensor_add(t2[:], t4[:, :, 0:2], t4[:, :, 2:4])
                                t1 = wpool.tile([128, SB], F32, tag=f"f1{t4.name[:3]}")
                                nc.vector.tensor_add(t1[:], t2[:, :, 0], t2[:, :, 1])
                                outs.append(t1)
                            x_un, den = outs
                            rden = wpool.tile([128, SB], F32, tag="rden")
                            nc.vector.reciprocal(rden[:], den[:])
                            nc.vector.tensor_mul(x_strip[:, nb0:nb0 + SB],
                                                 x_un[:], rden[:])

                    for s in range(NC // S):
                        n0 = s * S
                        if n0 % HNC == 0:
                            nbh = iopool.tile([128, HNC, K], BF16, tag="nbh")
                            nc.sync.dma_start(nbh[:], nbp[pp, :, n0:n0 + HNC, :])
                        nb_t = nbh[:, n0 % HNC:n0 % HNC + S, :]
                        # v projection, ScalarE evicts to SBUF bf16
                        v_ps = pspool.tile([128, S, K], F32, tag="ps")
                        for j in range(NCHUNK):
                            nc.tensor.matmul(
                                v_ps[:, j * PTS:(j + 1) * PTS, :], wv_t[:],
                                nb_t[:, j * PTS:(j + 1) * PTS, :],
                                start=True, stop=True)
                        v_sb = wpool.tile([128, S, K], ADT, tag="vsb")
                        nc.scalar.copy(v_sb[:], v_ps[:])
                        # k projection + q*k
                        k_ps = pspool.tile([128, S, K], F32, tag="ps")
                        for j in range(NCHUNK):
                            nc.tensor.matmul(
                                k_ps[:, j * PTS:(j + 1) * PTS, :], wk_t[:],
                                nb_t[:, j * PTS:(j + 1) * PTS, :],
                                start=True, stop=True)
                        prod = wpool.tile([128, S, K], BF16, tag="prod")
                        qb = q_sb[:, n0:n0 + S].unsqueeze(2).broadcast_to([128, S, K])
                        nc.vector.tensor_mul(prod[:], k_ps[:], qb)
                        # reduce over d + replicate -> exp
                        e_ps = pspool.tile([128, S, K], F32, tag="ps")
                        for j in range(NCHUNK):
                            nc.tensor.matmul(
                                e_ps[:, j * PTS:(j + 1) * PTS, :], wo_t[:],
                                prod[:, j * PTS:(j + 1) * PTS, :],
                                start=True, stop=True)
                        if n0 % SB == 0:
                            attn_b = wpool.tile([128, SB, K], ADT, tag="attn")
                            tx4 = wpool.tile([128, SB, 4], ADT, tag="tx4", name="tx4")
                            td4 = wpool.tile([128, SB, 4], ADT, tag="td4", name="td4")
                        o = n0 % SB
                        nc.scalar.activation(attn_b[:, o:o + S, :], e_ps[:],
                                             mybir.ActivationFunctionType.Exp)
                        if pend is not None:
                            back_half(pend)
                        pend = (v_sb, attn_b, n0, tx4, td4)
                    back_half(pend)
                    nc.sync.dma_start(xout[pp], x_strip[:])

            if R == 1:
                body()
            else:
                with tc.For_i(0, R, 1):
                    body()

    nc.compile()
    _nc_cache[key] = nc
    return nc


def prep_inputs(pcd, neighbors, q_w, k_w, v_w, NC=NC):
    """Host-side prep: cast to bf16, pair-stack batches, build stationaries."""
    bf = ml_dtypes.bfloat16
    s = 1.0 / np.sqrt(np.float32(D))
    qwT = (q_w.astype(np.float32) * s).T.astype(bf)  # (c, hd)
    kwT = k_w.T.astype(bf)
    vwT = v_w.T.astype(bf)

    def blockdiag(m):
        z = np.zeros((128, 128), dtype=bf)
        z[:64, :64] = m
        z[64:, 64:] = m
        return z

    wq = blockdiag(qwT)
    wk = blockdiag(kwT)
    wv = blockdiag(vwT)
    blk = np.kron(np.eye(H, dtype=np.float32), np.ones((D, D), np.float32))
    wones = blockdiag(blk.astype(bf))

    nbs = neighbors.reshape(2, 2 * CIN, N, K)    # (pair, bb*64+c, n, k)
    pcds = pcd.reshape(2, 2 * CIN, N)
    ncores = N // NC
    in_maps = []
    for i in range(ncores):
        sl = slice(i * NC, (i + 1) * NC)
        in_maps.append({
            "nbp": np.ascontiguousarray(nbs[:, :, sl, :]).astype(bf),
            "pcdp": np.ascontiguousarray(pcds[:, :, sl]).astype(bf),
            "wq": wq, "wk": wk, "wv": wv, "wones": wones,
        })
    return in_maps


def assemble_output(results, NC=NC):
    ncores = len(results)
    out = np.empty((B, COUT, N), dtype=np.float32)
    for i, r in enumerate(results):
        x = r["xout"].reshape(B, COUT, NC)  # (2,128,NC) -> (4,64,NC)
        out[:, :, i * NC:(i + 1) * NC] = x
    return out


BEST = dict(S=32, psum_bufs=3)


def kernel(pcd, neighbors, q_w, k_w, v_w):
    pcd = np.asarray(pcd, dtype=np.float32)
    neighbors = np.asarray(neighbors, dtype=np.float32)
    nc = build_nc(NC=NC, R=1, **BEST)
    in_maps = prep_inputs(pcd, neighbors, q_w, k_w, v_w)
    res = run_bass_kernel_spmd(nc, in_maps, core_ids=list(range(NCORES)))
    return assemble_output(res.results)


if __name__ == "__main__":
    rng = np.random.default_rng(0)
    ins = {
        "pcd": rng.standard_normal((B, CIN, N), dtype=np.float32),
        "neighbors": rng.standard_normal((B, CIN, N, K), dtype=np.float32),
        "q_w": (rng.standard_normal((COUT, CIN), dtype=np.float32) / 8.0),
        "k_w": (rng.standard_normal((COUT, CIN), dtype=np.float32) / 8.0),
        "v_w": (rng.standard_normal((COUT, CIN), dtype=np.float32) / 8.0),
    }
    out = kernel(**ins)
    print("kernel output", out.shape, out.dtype)
